# revision 9
# baseline (speedup 1.0000x reference)
"""Chamfer loss kernel for Trainium2 (8 NeuronCores, Bass/Tile).

Problem: x, y of shape [8192, 128] fp32.
  dist[i,j] = max(||x_i||^2 + ||y_j||^2 - 2 x_i.y_j, 0)
  loss = (sum_j min_i dist[i,j] + sum_i min_j dist[i,j]) / 8192

Sharding: x rows are split across the 8 cores (1024 rows each); every core
holds all of y. Each core computes its [1024, 8192] distance tile via PE
matmuls (K = 128 features on partitions):

  PSUM tile = (-2 x_chunk^T)^T @ y^T            (K=128 fp16 matmul)
            + [ones; x2_chunk]^T @ [y2; ones]   (K=2 rank-2 bias fold)
  => PSUM[i, j] = shifted dist (x2/y2 are shifted by their minima; the
     shift S is added back to the final [128]-sized min vectors, which
     keeps the fp16 bias rows small and precise).

ScalarE drains PSUM -> SBUF fp16 "E" tiles (one pass). VectorE then does
both reductions as 2x-mode fp16 tensor-tensor mins:
  - per-x row minima: pairwise halving tree along the free axis
  - per-y col minima: elementwise min accumulated across the 8 x-chunks
    (cross-partition reduction deferred to a DMA-transpose + halving tree)
Finally a byte-wise AllGather exchanges each core's per-y minima vector
[128,64] plus its local row-loss partial; every core reduces the gathered
data to the final scalar loss.
"""

import os
import sys

import numpy as np

sys.path.insert(0, "/opt/trn_rl_repo")
os.environ.setdefault("MYCRO_LOCAL_CACHE", "1")

import concourse.bass as bass
import concourse.bacc as bacc
import concourse.mybir as mybir
import concourse.tile as tile
from concourse.bass_utils import run_bass_kernel_spmd

FP16 = mybir.dt.float16
FP32 = mybir.dt.float32
AF = mybir.ActivationFunctionType
ALU = mybir.AluOpType

NPTS = 8192          # points in x and in y
DIM = 128            # feature dim = matmul contraction K
NCORES = 8
LOCAL = NPTS // NCORES   # 1024 x-rows per core
P = 128              # partitions
CHUNKS = LOCAL // P      # 8 chunks of 128 x-rows per core
JT = 512             # j-tile width (one PSUM bank of fp32)
GROUP = 4            # j-tiles per PSUM pool buffer / ACT drain
NGRP = NPTS // (JT * GROUP)  # 4 drain groups per chunk
NBLK = NPTS // P     # 64 column blocks of 128 y-points


def _build_module():
    nc = bacc.Bacc(
        "TRN2",
        target_bir_lowering=False,
        debug=False,
        num_devices=NCORES,
    )

    xT2 = nc.dram_tensor("xT2", [P, LOCAL], FP16, kind="ExternalInput")
    yT = nc.dram_tensor("yT", [P, NPTS], FP16, kind="ExternalInput")
    fold_lhsT = nc.dram_tensor("fold_lhsT", [2, LOCAL], FP16, kind="ExternalInput")
    fold_rhs = nc.dram_tensor("fold_rhs", [2, NPTS], FP16, kind="ExternalInput")
    shift = nc.dram_tensor("shift", [P, 1], FP32, kind="ExternalInput")
    loss = nc.dram_tensor("loss", [1, 1], FP32, kind="ExternalOutput")

    cc_in = nc.dram_tensor("cc_in", [P, NBLK + 1], FP32)
    cc_out = nc.dram_tensor("cc_out", [NCORES, P, NBLK + 1], FP32, addr_space="Shared")

    with tile.TileContext(nc) as tc:
        with (
            tc.tile_pool(name="const", bufs=1) as cpool,
            tc.tile_pool(name="big", bufs=1) as bigpool,
            tc.tile_pool(name="epool", bufs=2) as epool,
            tc.tile_pool(name="scratch", bufs=1) as spool,
        ):
            sb_xT2 = cpool.tile([P, LOCAL], FP16, tag="xT2")
            sb_yT = cpool.tile([P, NPTS], FP16, tag="yT")
            sb_flhs = cpool.tile([2, LOCAL], FP16, tag="flhs")
            sb_frhs = cpool.tile([2, NPTS], FP16, tag="frhs")
            sb_shift = cpool.tile([P, 1], FP32, tag="shift")
            sb_ones = cpool.tile([P, 1], FP32, tag="ones")

            nc.sync.dma_start(sb_xT2[:], xT2[:])
            nc.sync.dma_start(sb_yT[:], yT[:])
            nc.sync.dma_start(sb_flhs[:], fold_lhsT[:])
            nc.sync.dma_start(sb_frhs[:], fold_rhs[:])
            nc.sync.dma_start(sb_shift[:], shift[:])
            nc.vector.memset(sb_ones[:], 1.0)

            colacc = bigpool.tile([P, NPTS], FP16, tag="colacc")
            rowmins = spool.tile([P, CHUNKS], FP32, tag="rowmins")
            rtree = spool.tile([P, NPTS // 2], FP16, tag="rtree")

            with tc.tile_pool(name="psum", bufs=2, space="PSUM") as psum_pool:
                for c in range(CHUNKS):
                    e_c = epool.tile([P, NPTS], FP16, tag="E")
                    csl = bass.ts(c, P)
                    for g in range(NGRP):
                        pt = psum_pool.tile([P, GROUP * JT], FP32, tag="D")
                        # main matmuls of the group share one lhsT load;
                        # the K=2 bias folds share another.
                        for t in range(GROUP):
                            j0 = (g * GROUP + t) * JT
                            nc.tensor.matmul(
                                pt[:, bass.ts(t, JT)],
                                lhsT=sb_xT2[:, csl],
                                rhs=sb_yT[:, j0 : j0 + JT],
                                start=True,
                                stop=False,
                            )
                        for t in range(GROUP):
                            j0 = (g * GROUP + t) * JT
                            nc.tensor.matmul(
                                pt[:, bass.ts(t, JT)],
                                lhsT=sb_flhs[:, csl],
                                rhs=sb_frhs[:, j0 : j0 + JT],
                                start=False,
                                stop=True,
                            )
                        gsl = bass.ts(g, GROUP * JT)
                        nc.scalar.activation(e_c[:, gsl], pt[:], AF.Copy)

                    # per-y minima accumulated elementwise across chunks
                    if c == 0:
                        nc.vector.tensor_copy(colacc[:], e_c[:])
                    else:
                        nc.vector.tensor_tensor(
                            colacc[:], e_c[:], colacc[:], op=ALU.min
                        )
                    # per-x minima: halving tree along the free axis
                    half = NPTS // 2
                    nc.vector.tensor_tensor(
                        rtree[:, 0:half], e_c[:, 0:half], e_c[:, half:NPTS], op=ALU.min
                    )
                    while half > JT:
                        h2 = half // 2
                        nc.vector.tensor_tensor(
                            rtree[:, 0:h2], rtree[:, 0:h2], rtree[:, h2:half],
                            op=ALU.min,
                        )
                        half = h2
                    nc.vector.tensor_reduce(
                        rowmins[:, c : c + 1], rtree[:, 0:JT],
                        axis=mybir.AxisListType.X, op=ALU.min,
                    )

            # ---- tail: finalize local partials -------------------------
            # rowmins -> + S, clamp 0, sum over local rows -> scalar S1_c
            rclamp = spool.tile([P, CHUNKS], FP32, tag="rclamp")
            nc.vector.tensor_scalar(
                rclamp[:], rowmins[:], sb_shift[:], 0.0,
                op0=ALU.add, op1=ALU.max,
            )
            rsum = spool.tile([P, 1], FP32, tag="rsum")
            nc.vector.tensor_reduce(
                rsum[:], rclamp[:], axis=mybir.AxisListType.X, op=ALU.add
            )

            # per-y minima: block transpose then halving tree over the
            # former partition axis (still in shifted space, fp16)
            colaccT = bigpool.tile([P, NBLK, P], FP16, tag="colaccT")
            nc.sync.dma_start_transpose(colaccT[:], colacc[:])
            w = P // 2
            ctree = bigpool.tile([P, NBLK, P // 2], FP16, tag="ctree")
            nc.vector.tensor_tensor(
                ctree[:, :, 0:w], colaccT[:, :, 0:w], colaccT[:, :, w:P], op=ALU.min
            )
            while w > 1:
                h2 = w // 2
                nc.vector.tensor_tensor(
                    ctree[:, :, 0:h2], ctree[:, :, 0:h2], ctree[:, :, h2:w],
                    op=ALU.min,
                )
                w = h2
            colmin = spool.tile([P, NBLK], FP32, tag="colmin")
            nc.vector.tensor_copy(colmin[:], ctree[:, :, 0])

            with tc.tile_pool(name="psum2", bufs=1, space="PSUM") as pp2:
                ps_r = pp2.tile([1, 1], FP32, tag="ps_r")
                nc.tensor.matmul(ps_r[:], lhsT=rsum[:], rhs=sb_ones[:], start=True,
                                 stop=True)
                s1_c = spool.tile([1, 1], FP32, tag="s1c")
                nc.scalar.activation(s1_c[:], ps_r[:], AF.Copy)

                # exchange: [colmin fp32 | S1_c] via byte-wise AllGather
                nc.sync.dma_start(cc_in[:, 0:NBLK], colmin[:])
                nc.sync.dma_start(cc_in[0:1, NBLK : NBLK + 1], s1_c[:])
                nc.gpsimd.collective_compute(
                    "AllGather",
                    ALU.bypass,
                    replica_groups=[list(range(NCORES))],
                    ins=[cc_in[:]],
                    outs=[cc_out[:]],
                )

                # ---- global reduction (identical on every core) --------
                gbuf = spool.tile([P, NCORES, NBLK + 1], FP32, tag="gbuf")
                nc.sync.dma_start(
                    gbuf[:], cc_out.ap().rearrange("c p n -> p c n")
                )
                gmin = spool.tile([P, NCORES // 2, NBLK], FP32, tag="gmin")
                cw = NCORES // 2
                nc.vector.tensor_tensor(
                    gmin[:, 0:cw, :], gbuf[:, 0:cw, 0:NBLK],
                    gbuf[:, cw:NCORES, 0:NBLK], op=ALU.min,
                )
                while cw > 1:
                    h2 = cw // 2
                    nc.vector.tensor_tensor(
                        gmin[:, 0:h2, :], gmin[:, 0:h2, :], gmin[:, h2:cw, :],
                        op=ALU.min,
                    )
                    cw = h2
                gclamp = spool.tile([P, NBLK], FP32, tag="gclamp")
                nc.vector.tensor_scalar(
                    gclamp[:], gmin[:, 0, :], sb_shift[:], 0.0,
                    op0=ALU.add, op1=ALU.max,
                )
                csum = spool.tile([P, 1], FP32, tag="csum")
                nc.vector.tensor_reduce(
                    csum[:], gclamp[:], axis=mybir.AxisListType.X, op=ALU.add
                )
                ps_c = pp2.tile([1, 1], FP32, tag="ps_c")
                nc.tensor.matmul(ps_c[:], lhsT=csum[:], rhs=sb_ones[:], start=True,
                                 stop=True)

                s1sum = spool.tile([1, 1], FP32, tag="s1sum")
                nc.vector.tensor_reduce(
                    s1sum[:], gbuf[0:1, :, NBLK], axis=mybir.AxisListType.X,
                    op=ALU.add,
                )
                tot = spool.tile([1, 1], FP32, tag="tot")
                nc.vector.tensor_tensor(tot[:], ps_c[:], s1sum[:], op=ALU.add)
                lres = spool.tile([1, 1], FP32, tag="lres")
                nc.vector.tensor_scalar_mul(lres[:], tot[:], 1.0 / NPTS)
                nc.sync.dma_start(loss[:], lres[:])

    nc.compile()
    return nc


_NC_CACHE = None


def _get_module():
    global _NC_CACHE
    if _NC_CACHE is None:
        _NC_CACHE = _build_module()
    return _NC_CACHE


_RUNNER_CACHE = None


def _get_runner():
    """Build (once) a jitted SPMD callable over the 8 cores.

    Mirrors concourse.bass2jax.run_bass_via_pjrt but caches the jitted
    function so repeated calls don't re-trace, and exposes the pieces
    needed for device-resident benchmarking.
    """
    global _RUNNER_CACHE
    if _RUNNER_CACHE is not None:
        return _RUNNER_CACHE

    import jax
    from jax.sharding import Mesh, PartitionSpec
    from jax.experimental.shard_map import shard_map
    import concourse.mybir as _mybir
    from concourse import bass2jax

    nc = _get_module()
    bass2jax.install_neuronx_cc_hook()

    partition_name = (
        nc.partition_id_tensor.name if nc.partition_id_tensor else None
    )
    in_names: list[str] = []
    out_names: list[str] = []
    out_avals: list[jax.core.ShapedArray] = []
    zero_outs: list[np.ndarray] = []
    for alloc in nc.m.functions[0].allocations:
        if not isinstance(alloc, _mybir.MemoryLocationSet):
            continue
        name = alloc.memorylocations[0].name
        if alloc.kind == "ExternalInput":
            if name != partition_name:
                in_names.append(name)
        elif alloc.kind == "ExternalOutput":
            out_names.append(name)
            shape = tuple(alloc.tensor_shape)
            dtype = _mybir.dt.np(alloc.dtype)
            out_avals.append(jax.core.ShapedArray(shape, dtype))
            zero_outs.append(np.zeros(shape, dtype))
    n_params = len(in_names)
    n_outs = len(out_avals)
    all_names = in_names + out_names
    if partition_name is not None:
        all_names = all_names + [partition_name]

    def _body(*args):
        operands = list(args)
        if partition_name is not None:
            operands.append(bass2jax.partition_id_tensor())
        outs = bass2jax._bass_exec_p.bind(
            *operands,
            out_avals=tuple(out_avals),
            in_names=tuple(all_names),
            out_names=tuple(out_names),
            lowering_input_output_aliases=(),
            sim_require_finite=True,
            sim_require_nnan=True,
            nc=nc,
        )
        return tuple(outs)

    devices = jax.devices()[:NCORES]
    mesh = Mesh(np.asarray(devices), ("core",))
    in_specs = (PartitionSpec("core"),) * (n_params + n_outs)
    out_specs = (PartitionSpec("core"),) * n_outs
    donate = tuple(range(n_params, n_params + n_outs))
    sharded = jax.jit(
        shard_map(_body, mesh=mesh, in_specs=in_specs, out_specs=out_specs,
                  check_rep=False),
        donate_argnums=donate,
        keep_unused=True,
    )
    _RUNNER_CACHE = (sharded, in_names, out_names, out_avals, zero_outs, mesh)
    return _RUNNER_CACHE


def _run(in_maps):
    sharded, in_names, out_names, out_avals, zero_outs, _ = _get_runner()
    concat_in = [
        np.concatenate([np.asarray(in_maps[c][n]) for c in range(NCORES)], axis=0)
        for n in in_names
    ]
    concat_zeros = [
        np.zeros((NCORES * z.shape[0], *z.shape[1:]), z.dtype) for z in zero_outs
    ]
    out_arrs = sharded(*concat_in, *concat_zeros)
    return [
        {
            n: np.asarray(out_arrs[i]).reshape(NCORES, *out_avals[i].shape)[c]
            for i, n in enumerate(out_names)
        }
        for c in range(NCORES)
    ]


def _prep_inputs(x: np.ndarray, y: np.ndarray):
    x = np.asarray(x, np.float32)
    y = np.asarray(y, np.float32)
    x2 = np.sum(x.astype(np.float64) ** 2, axis=1)
    y2 = np.sum(y.astype(np.float64) ** 2, axis=1)
    s = float(x2.min() + y2.min())
    x2s = (x2 - x2.min()).astype(np.float32)
    y2s = (y2 - y2.min()).astype(np.float32)

    yT = np.ascontiguousarray(y.T).astype(np.float16)
    fold_rhs = np.empty((2, NPTS), np.float16)
    fold_rhs[0] = y2s.astype(np.float16)
    fold_rhs[1] = 1.0
    shift = np.full((P, 1), s, np.float32)

    in_maps = []
    for c in range(NCORES):
        sl = slice(c * LOCAL, (c + 1) * LOCAL)
        xT2 = np.ascontiguousarray((-2.0 * x[sl]).T).astype(np.float16)
        fold_lhsT = np.empty((2, LOCAL), np.float16)
        fold_lhsT[0] = 1.0
        fold_lhsT[1] = x2s[sl].astype(np.float16)
        in_maps.append(
            {
                "xT2": xT2,
                "yT": yT,
                "fold_lhsT": fold_lhsT,
                "fold_rhs": fold_rhs,
                "shift": shift,
            }
        )
    return in_maps


def kernel(x: np.ndarray, y: np.ndarray, **_ignored):
    in_maps = _prep_inputs(x, y)
    results = _run(in_maps)
    return np.float32(results[0]["loss"][0, 0])


# revision 48
# speedup vs baseline: 30.1094x; 30.1094x over previous
"""Chamfer loss kernel for Trainium2 (8 NeuronCores, Bass/Tile).

Problem: x, y of shape [8192, 128] fp32.
  dist[i,j] = max(||x_i||^2 + ||y_j||^2 - 2 x_i.y_j, 0)
  loss = (sum_j min_i dist[i,j] + sum_i min_j dist[i,j]) / 8192

Sharding: x rows are split across the 8 cores (1024 rows each); every core
holds all of y. Each core computes its [1024, 8192] distance tile via PE
matmuls (K = 128 features on partitions):

  PSUM tile = (-2 x_chunk^T)^T @ y^T            (K=128 fp16 matmul)
            + [ones; x2_chunk]^T @ [y2; ones]   (K=2 rank-2 bias fold)
  => PSUM[i, j] = shifted dist (x2/y2 are shifted by their minima; the
     shift S is added back to the final [128]-sized min vectors, which
     keeps the fp16 bias rows small and precise).

ScalarE drains PSUM -> SBUF fp16 "E" tiles (one pass). VectorE then does
both reductions as 2x-mode fp16 tensor-tensor mins:
  - per-x row minima: pairwise halving tree along the free axis
  - per-y col minima: elementwise min accumulated across the 8 x-chunks
    (cross-partition reduction deferred to a DMA-transpose + halving tree)
Finally a byte-wise AllGather exchanges each core's per-y minima vector
[128,64] plus its local row-loss partial; every core reduces the gathered
data to the final scalar loss.
"""

import os
import sys

import numpy as np

sys.path.insert(0, "/opt/trn_rl_repo")
os.environ.setdefault("MYCRO_LOCAL_CACHE", "1")

import concourse.bass as bass
import concourse.bacc as bacc
import concourse.mybir as mybir
import concourse.tile as tile
from concourse.bass_utils import run_bass_kernel_spmd

FP16 = mybir.dt.float16
FP32 = mybir.dt.float32
AF = mybir.ActivationFunctionType
ALU = mybir.AluOpType

NPTS = 8192          # points in x and in y
DIM = 128            # feature dim = matmul contraction K
NCORES = 8
LOCAL = NPTS // NCORES   # 1024 x-rows per core
P = 128              # partitions
CHUNKS = LOCAL // P      # 8 chunks of 128 x-rows per core
JT = 512             # j-tile width (one PSUM bank of fp32)
GROUP = 4            # j-tiles per PSUM pool buffer / ACT drain
NGRP = NPTS // (JT * GROUP)  # 4 drain groups per chunk
NBLK = NPTS // P     # 64 column blocks of 128 y-points


EBUFS = int(os.environ.get("K_EBUFS", "3"))
COLACC_GRAIN = int(os.environ.get("K_COLACC_GRAIN", "2048"))
RTREE_FINE = int(os.environ.get("K_RTREE_FINE", "0"))
SPLITCOL = int(os.environ.get("K_SPLITCOL", "0"))
SLICELOAD = int(os.environ.get("K_SLICELOAD", "0"))
REPEAT = int(os.environ.get("K_REPEAT", "1"))  # 'main' stage only


def _build_module(stage: str = "full"):
    """stage: 'full' | 'nocc' (skip collective+global) | 'main' (chunk loop
    only) | 'pe' (matmuls only) | 'pedrain' (matmuls + ACT drain)."""
    nc = bacc.Bacc(
        "TRN2",
        target_bir_lowering=False,
        debug=False,
        num_devices=NCORES,
    )

    xT2 = nc.dram_tensor("xT2", [P, LOCAL], FP16, kind="ExternalInput")
    yT = nc.dram_tensor("yT", [P, NPTS], FP16, kind="ExternalInput")
    fold_lhsT = nc.dram_tensor("fold_lhsT", [2, LOCAL], FP16, kind="ExternalInput")
    fold_rhs = nc.dram_tensor("fold_rhs", [2, NPTS], FP16, kind="ExternalInput")
    shift = nc.dram_tensor("shift", [P, 1], FP32, kind="ExternalInput")
    if stage == "host":
        # per-core partials: cols 0..63 = per-y minima (shifted space, no
        # clamp — the global min across cores happens on the host), col 64
        # = per-partition row-loss sum (clamped, final for this core's rows)
        loss = nc.dram_tensor("parts", [P, NBLK + 1], FP32,
                              kind="ExternalOutput")
        cc_in = cc_out = None
    else:
        loss = nc.dram_tensor("loss", [1, 1], FP32, kind="ExternalOutput")
        cc_in = nc.dram_tensor("cc_in", [P, NBLK + 1], FP32)
        cc_out = nc.dram_tensor("cc_out", [NCORES, P, NBLK + 1], FP32,
                                addr_space="Shared")

    with tile.TileContext(nc) as tc:
        with (
            tc.tile_pool(name="const", bufs=1) as cpool,
            tc.tile_pool(name="big", bufs=1) as bigpool,
            tc.tile_pool(name="epool", bufs=EBUFS) as epool,
            tc.tile_pool(name="scratch", bufs=1) as spool,
        ):
            sb_xT2 = cpool.tile([P, LOCAL], FP16, tag="xT2")
            sb_yT = cpool.tile([P, NPTS], FP16, tag="yT")
            sb_flhs = cpool.tile([2, LOCAL], FP16, tag="flhs")
            sb_frhs = cpool.tile([2, NPTS], FP16, tag="frhs")
            sb_shift = cpool.tile([P, 1], FP32, tag="shift")
            sb_ones = cpool.tile([P, 1], FP32, tag="ones")

            # sliced loads so early matmuls only wait on their own slice
            if SLICELOAD:
                for t in range(NPTS // JT):
                    sl = bass.ts(t, JT)
                    nc.sync.dma_start(sb_yT[:, sl], yT[:, sl])
                for c in range(CHUNKS):
                    sl = bass.ts(c, P)
                    nc.sync.dma_start(sb_xT2[:, sl], xT2[:, sl])
            else:
                nc.sync.dma_start(sb_yT[:], yT[:])
                nc.sync.dma_start(sb_xT2[:], xT2[:])
            nc.sync.dma_start(sb_flhs[:], fold_lhsT[:])
            nc.sync.dma_start(sb_frhs[:], fold_rhs[:])
            nc.sync.dma_start(sb_shift[:], shift[:])
            nc.vector.memset(sb_ones[:], 1.0)

            # two column-min accumulators: A over chunks 0-3, B over 4-7.
            # A finalizes (transpose + partition-tree) while B's chunks
            # still run, hiding half the tail.
            colaccA = bigpool.tile([P, NPTS], FP16, tag="colaccA")
            colaccB = bigpool.tile([P, NPTS], FP16, tag="colaccB")
            colaccT = bigpool.tile([P, NBLK, P], FP16, tag="colaccT")
            ctree = bigpool.tile([P, NBLK, P // 2], FP16, tag="ctree")
            cminA = spool.tile([P, NBLK], FP16, tag="cminA")
            rowmins = spool.tile([P, CHUNKS], FP32, tag="rowmins")
            rtree = spool.tile([P, NPTS // 2], FP16, tag="rtree")

            def _ctree_reduce(dst_small, b0=0, b1=NBLK):
                """Halving min-tree over colaccT's last axis for blocks
                [b0:b1]; result copied to dst_small[:, b0:b1]."""
                w = P // 2
                nc.vector.tensor_tensor(
                    ctree[:, b0:b1, 0:w], colaccT[:, b0:b1, 0:w],
                    colaccT[:, b0:b1, w:P], op=ALU.min,
                )
                while w > 1:
                    h2 = w // 2
                    nc.vector.tensor_tensor(
                        ctree[:, b0:b1, 0:h2], ctree[:, b0:b1, 0:h2],
                        ctree[:, b0:b1, h2:w], op=ALU.min,
                    )
                    w = h2
                nc.vector.tensor_copy(dst_small[:, b0:b1], ctree[:, b0:b1, 0])

            nrep = REPEAT if stage in ("main", "host") else 1
            with tc.tile_pool(name="psum", bufs=2, space="PSUM") as psum_pool:
              for rep in range(nrep):
                for c in range(CHUNKS):
                    e_c = epool.tile([P, NPTS], FP16, tag="E")
                    csl = bass.ts(c, P)
                    for g in range(NGRP):
                        pt = psum_pool.tile([P, GROUP * JT], FP32, tag="D")
                        # main matmuls of the group share one lhsT load;
                        # the K=2 bias folds share another.
                        for t in range(GROUP):
                            j0 = (g * GROUP + t) * JT
                            nc.tensor.matmul(
                                pt[:, bass.ts(t, JT)],
                                lhsT=sb_xT2[:, csl],
                                rhs=sb_yT[:, j0 : j0 + JT],
                                start=True,
                                stop=False,
                            )
                        for t in range(GROUP):
                            j0 = (g * GROUP + t) * JT
                            nc.tensor.matmul(
                                pt[:, bass.ts(t, JT)],
                                lhsT=sb_flhs[:, csl],
                                rhs=sb_frhs[:, j0 : j0 + JT],
                                start=False,
                                stop=True,
                            )
                        gsl = bass.ts(g, GROUP * JT)
                        if stage == "pe":
                            # keep a consumer so matmuls aren't dead: tiny
                            # copy of one column per group
                            nc.scalar.activation(
                                e_c[:, g : g + 1], pt[:, 0:1], AF.Copy
                            )
                        else:
                            nc.scalar.activation(e_c[:, gsl], pt[:], AF.Copy)

                    if stage in ("pe", "pedrain"):
                        # tiny reader keeps each chunk's work live
                        nc.vector.tensor_copy(
                            rowmins[0:1, c : c + 1], e_c[0:1, 0:1]
                        )
                        continue
                    # per-y minima accumulated elementwise across chunks
                    if SPLITCOL:
                        colacc = colaccA if c < CHUNKS // 2 else colaccB
                        first = c % (CHUNKS // 2) == 0
                    else:
                        colacc = colaccA
                        first = c == 0
                    for j0 in range(0, NPTS, COLACC_GRAIN):
                        sl = slice(j0, j0 + COLACC_GRAIN)
                        if first:
                            nc.vector.tensor_copy(colacc[:, sl], e_c[:, sl])
                        else:
                            nc.vector.tensor_tensor(
                                colacc[:, sl], e_c[:, sl], colacc[:, sl],
                                op=ALU.min,
                            )
                    if (SPLITCOL and c == CHUNKS // 2 - 1
                            and stage not in ("pe", "pedrain", "main")):
                        # finalize A while B's chunks still run
                        nc.sync.dma_start_transpose(colaccT[:], colaccA[:])
                        _ctree_reduce(cminA)
                    # per-x minima: halving tree along the free axis
                    half = NPTS // 2
                    if RTREE_FINE:
                        # first level in two group-pair halves for overlap
                        q = NPTS // 4
                        nc.vector.tensor_tensor(
                            rtree[:, 0:q], e_c[:, 0:q], e_c[:, half : half + q],
                            op=ALU.min,
                        )
                        nc.vector.tensor_tensor(
                            rtree[:, q:half], e_c[:, q:half], e_c[:, half + q :],
                            op=ALU.min,
                        )
                    else:
                        nc.vector.tensor_tensor(
                            rtree[:, 0:half], e_c[:, 0:half], e_c[:, half:NPTS],
                            op=ALU.min,
                        )
                    while half > JT:
                        h2 = half // 2
                        nc.vector.tensor_tensor(
                            rtree[:, 0:h2], rtree[:, 0:h2], rtree[:, h2:half],
                            op=ALU.min,
                        )
                        half = h2
                    nc.vector.tensor_reduce(
                        rowmins[:, c : c + 1], rtree[:, 0:JT],
                        axis=mybir.AxisListType.X, op=ALU.min,
                    )
                if stage == "host":
                    _emit_tail(nc, tc, stage, spool,
                               colaccB if SPLITCOL else colaccA, colaccT,
                               ctree, cminA if SPLITCOL else None,
                               _ctree_reduce, rowmins, sb_shift, sb_ones,
                               cc_in, cc_out, loss)

            if stage in ("main", "pe", "pedrain"):
                lres0 = spool.tile([1, 1], FP32, tag="lres0")
                nc.vector.tensor_copy(lres0[:], rowmins[0:1, 0:1])
                nc.sync.dma_start(loss[:], lres0[:])
            elif stage != "host":
                _emit_tail(nc, tc, stage, spool,
                           colaccB if SPLITCOL else colaccA, colaccT, ctree,
                           cminA if SPLITCOL else None, _ctree_reduce,
                           rowmins, sb_shift, sb_ones, cc_in, cc_out, loss)

    nc.compile()
    return nc


def _emit_tail(nc, tc, stage, spool, colaccB, colaccT, ctree, cminA,
               _ctree_reduce, rowmins, sb_shift, sb_ones, cc_in, cc_out, loss):
    # rowmins -> + S, clamp 0, sum over local rows -> scalar S1_c
    rclamp = spool.tile([P, CHUNKS], FP32, tag="rclamp")
    nc.vector.tensor_scalar(
        rclamp[:], rowmins[:], sb_shift[:], 0.0, op0=ALU.add, op1=ALU.max
    )
    rsum = spool.tile([P, 1], FP32, tag="rsum")
    nc.vector.tensor_reduce(
        rsum[:], rclamp[:], axis=mybir.AxisListType.X, op=ALU.add
    )

    # finalize B's per-y minima, combine with A's.  The transpose runs in
    # two j-halves so the partition-tree on half 0 overlaps the DMA of
    # half 1.
    cminB = spool.tile([P, NBLK], FP16, tag="cminB")
    half = NPTS // 2
    hb = NBLK // 2
    nc.sync.dma_start_transpose(colaccT[:, 0:hb, :], colaccB[:, 0:half])
    nc.sync.dma_start_transpose(colaccT[:, hb:NBLK, :], colaccB[:, half:NPTS])
    _ctree_reduce(cminB, 0, hb)
    _ctree_reduce(cminB, hb, NBLK)
    colmin = spool.tile([P, NBLK], FP32, tag="colmin")
    if cminA is not None:
        nc.vector.tensor_tensor(colmin[:], cminA[:], cminB[:], op=ALU.min)
    else:
        nc.vector.tensor_copy(colmin[:], cminB[:])

    if stage == "host":
        nc.sync.dma_start(loss[:, 0:NBLK], colmin[:])
        nc.sync.dma_start(loss[:, NBLK : NBLK + 1], rsum[:])
        return

    with tc.tile_pool(name="psum2", bufs=1, space="PSUM") as pp2:
        ps_r = pp2.tile([1, 1], FP32, tag="ps_r")
        nc.tensor.matmul(ps_r[:], lhsT=rsum[:], rhs=sb_ones[:], start=True,
                         stop=True)
        s1_c = spool.tile([1, 1], FP32, tag="s1c")
        nc.scalar.activation(s1_c[:], ps_r[:], AF.Copy)

        if stage == "nocc":
            tot0 = spool.tile([1, 1], FP32, tag="tot0")
            nc.vector.tensor_tensor(tot0[:], s1_c[:], colmin[0:1, 0:1],
                                    op=ALU.add)
            nc.sync.dma_start(loss[:], tot0[:])
            return

        # exchange: [colmin fp32 | S1_c] via byte-wise AllGather
        nc.sync.dma_start(cc_in[:, 0:NBLK], colmin[:])
        nc.sync.dma_start(cc_in[0:1, NBLK : NBLK + 1], s1_c[:])
        nreps = REPEAT if stage == "fullrep" else 1
        for _ in range(nreps):
            nc.gpsimd.collective_compute(
                "AllGather",
                ALU.bypass,
                replica_groups=[list(range(NCORES))],
                ins=[cc_in[:]],
                outs=[cc_out[:]],
            )

        # ---- global reduction (identical on every core) ---------------
        gbuf = spool.tile([P, NCORES, NBLK + 1], FP32, tag="gbuf")
        nc.sync.dma_start(gbuf[:], cc_out.ap().rearrange("c p n -> p c n"))
        gmin = spool.tile([P, NCORES // 2, NBLK], FP32, tag="gmin")
        cw = NCORES // 2
        nc.vector.tensor_tensor(
            gmin[:, 0:cw, :], gbuf[:, 0:cw, 0:NBLK], gbuf[:, cw:NCORES, 0:NBLK],
            op=ALU.min,
        )
        while cw > 1:
            h2 = cw // 2
            nc.vector.tensor_tensor(
                gmin[:, 0:h2, :], gmin[:, 0:h2, :], gmin[:, h2:cw, :],
                op=ALU.min,
            )
            cw = h2
        gclamp = spool.tile([P, NBLK], FP32, tag="gclamp")
        nc.vector.tensor_scalar(
            gclamp[:], gmin[:, 0, :], sb_shift[:], 0.0, op0=ALU.add, op1=ALU.max
        )
        csum = spool.tile([P, 1], FP32, tag="csum")
        nc.vector.tensor_reduce(
            csum[:], gclamp[:], axis=mybir.AxisListType.X, op=ALU.add
        )
        ps_c = pp2.tile([1, 1], FP32, tag="ps_c")
        nc.tensor.matmul(ps_c[:], lhsT=csum[:], rhs=sb_ones[:], start=True,
                         stop=True)

        s1sum = spool.tile([1, 1], FP32, tag="s1sum")
        nc.vector.tensor_reduce(
            s1sum[:], gbuf[0:1, :, NBLK], axis=mybir.AxisListType.X, op=ALU.add
        )
        tot = spool.tile([1, 1], FP32, tag="tot")
        nc.vector.tensor_tensor(tot[:], ps_c[:], s1sum[:], op=ALU.add)
        lres = spool.tile([1, 1], FP32, tag="lres")
        nc.vector.tensor_scalar_mul(lres[:], tot[:], 1.0 / NPTS)
        nc.sync.dma_start(loss[:], lres[:])


_NC_CACHE: dict = {}


def _get_module(stage: str = "full"):
    if stage not in _NC_CACHE:
        _NC_CACHE[stage] = _build_module(stage)
    return _NC_CACHE[stage]


_RUNNER_CACHE: dict = {}


def _get_runner(stage: str = "full", donate: bool = True):
    """Build (once) a jitted SPMD callable over the 8 cores.

    Mirrors concourse.bass2jax.run_bass_via_pjrt but caches the jitted
    function so repeated calls don't re-trace, and exposes the pieces
    needed for device-resident benchmarking.
    """
    key = (stage, donate)
    if key in _RUNNER_CACHE:
        return _RUNNER_CACHE[key]

    import jax
    from jax.sharding import Mesh, PartitionSpec
    from jax.experimental.shard_map import shard_map
    import concourse.mybir as _mybir
    from concourse import bass2jax

    nc = _get_module(stage)
    bass2jax.install_neuronx_cc_hook()

    partition_name = (
        nc.partition_id_tensor.name if nc.partition_id_tensor else None
    )
    in_names: list[str] = []
    out_names: list[str] = []
    out_avals: list[jax.core.ShapedArray] = []
    zero_outs: list[np.ndarray] = []
    for alloc in nc.m.functions[0].allocations:
        if not isinstance(alloc, _mybir.MemoryLocationSet):
            continue
        name = alloc.memorylocations[0].name
        if alloc.kind == "ExternalInput":
            if name != partition_name:
                in_names.append(name)
        elif alloc.kind == "ExternalOutput":
            out_names.append(name)
            shape = tuple(alloc.tensor_shape)
            dtype = _mybir.dt.np(alloc.dtype)
            out_avals.append(jax.core.ShapedArray(shape, dtype))
            zero_outs.append(np.zeros(shape, dtype))
    n_params = len(in_names)
    n_outs = len(out_avals)
    all_names = in_names + out_names
    if partition_name is not None:
        all_names = all_names + [partition_name]

    def _body(*args):
        operands = list(args)
        if partition_name is not None:
            operands.append(bass2jax.partition_id_tensor())
        outs = bass2jax._bass_exec_p.bind(
            *operands,
            out_avals=tuple(out_avals),
            in_names=tuple(all_names),
            out_names=tuple(out_names),
            lowering_input_output_aliases=(),
            sim_require_finite=True,
            sim_require_nnan=True,
            nc=nc,
        )
        return tuple(outs)

    devices = jax.devices()[:NCORES]
    mesh = Mesh(np.asarray(devices), ("core",))
    in_specs = (PartitionSpec("core"),) * (n_params + n_outs)
    out_specs = (PartitionSpec("core"),) * n_outs
    jit_kw = (
        dict(donate_argnums=tuple(range(n_params, n_params + n_outs)))
        if donate
        else {}
    )
    sharded = jax.jit(
        shard_map(_body, mesh=mesh, in_specs=in_specs, out_specs=out_specs,
                  check_rep=False),
        keep_unused=True,
        **jit_kw,
    )
    _RUNNER_CACHE[key] = (sharded, in_names, out_names, out_avals, zero_outs,
                          mesh)
    return _RUNNER_CACHE[key]


def _run(in_maps, stage="full"):
    sharded, in_names, out_names, out_avals, zero_outs, _ = _get_runner(stage)
    concat_in = [
        np.concatenate([np.asarray(in_maps[c][n]) for c in range(NCORES)], axis=0)
        for n in in_names
    ]
    concat_zeros = [
        np.zeros((NCORES * z.shape[0], *z.shape[1:]), z.dtype) for z in zero_outs
    ]
    out_arrs = sharded(*concat_in, *concat_zeros)
    return [
        {
            n: np.asarray(out_arrs[i]).reshape(NCORES, *out_avals[i].shape)[c]
            for i, n in enumerate(out_names)
        }
        for c in range(NCORES)
    ]


def _prep_inputs(x: np.ndarray, y: np.ndarray):
    return _prep_inputs_s(x, y)[0]


def _prep_inputs_s(x: np.ndarray, y: np.ndarray):
    x = np.asarray(x, np.float32)
    y = np.asarray(y, np.float32)
    x2 = np.sum(x.astype(np.float64) ** 2, axis=1)
    y2 = np.sum(y.astype(np.float64) ** 2, axis=1)
    s = float(x2.min() + y2.min())
    x2s = (x2 - x2.min()).astype(np.float32)
    y2s = (y2 - y2.min()).astype(np.float32)

    yT = np.ascontiguousarray(y.T).astype(np.float16)
    fold_rhs = np.empty((2, NPTS), np.float16)
    fold_rhs[0] = y2s.astype(np.float16)
    fold_rhs[1] = 1.0
    shift = np.full((P, 1), s, np.float32)

    in_maps = []
    for c in range(NCORES):
        sl = slice(c * LOCAL, (c + 1) * LOCAL)
        xT2 = np.ascontiguousarray((-2.0 * x[sl]).T).astype(np.float16)
        fold_lhsT = np.empty((2, LOCAL), np.float16)
        fold_lhsT[0] = 1.0
        fold_lhsT[1] = x2s[sl].astype(np.float16)
        in_maps.append(
            {
                "xT2": xT2,
                "yT": yT,
                "fold_lhsT": fold_lhsT,
                "fold_rhs": fold_rhs,
                "shift": shift,
            }
        )
    return in_maps, s


def kernel(x: np.ndarray, y: np.ndarray, **_ignored):
    x = np.asarray(x, np.float32)
    y = np.asarray(y, np.float32)
    in_maps, s = _prep_inputs_s(x, y)
    results = _run(in_maps, stage="host")
    parts = np.stack([results[c]["parts"] for c in range(NCORES)])  # [8,128,65]
    colmin = parts[:, :, 0:NBLK].min(axis=0)       # global per-y minima
    s2 = np.maximum(colmin.astype(np.float64) + s, 0.0).sum()
    s1 = parts[:, :, NBLK].astype(np.float64).sum()
    return np.float32((s1 + s2) / NPTS)


# revision 50
# speedup vs baseline: 37.2988x; 1.2388x over previous
"""Chamfer loss kernel for Trainium2 (8 NeuronCores, Bass/Tile).

Problem: x, y of shape [8192, 128] fp32.
  dist[i,j] = max(||x_i||^2 + ||y_j||^2 - 2 x_i.y_j, 0)
  loss = (sum_j min_i dist[i,j] + sum_i min_j dist[i,j]) / 8192

Sharding: x rows are split across the 8 cores (1024 rows each); every core
holds all of y. Each core computes its [1024, 8192] distance tile via PE
matmuls (K = 128 features on partitions):

  PSUM tile = (-2 x_chunk^T)^T @ y^T            (K=128 fp16 matmul)
            + [ones; x2_chunk]^T @ [y2; ones]   (K=2 rank-2 bias fold)
  => PSUM[i, j] = shifted dist (x2/y2 are shifted by their minima; the
     shift S is added back to the final [128]-sized min vectors, which
     keeps the fp16 bias rows small and precise).

ScalarE drains PSUM -> SBUF fp16 "E" tiles (one pass). VectorE then does
both reductions as 2x-mode fp16 tensor-tensor mins:
  - per-x row minima: pairwise halving tree along the free axis
  - per-y col minima: elementwise min accumulated across the 8 x-chunks
    (cross-partition reduction deferred to a DMA-transpose + halving tree)
Finally a byte-wise AllGather exchanges each core's per-y minima vector
[128,64] plus its local row-loss partial; every core reduces the gathered
data to the final scalar loss.
"""

import os
import sys

import numpy as np

sys.path.insert(0, "/opt/trn_rl_repo")
os.environ.setdefault("MYCRO_LOCAL_CACHE", "1")

import concourse.bass as bass
import concourse.bacc as bacc
import concourse.mybir as mybir
import concourse.tile as tile
from concourse.bass_utils import run_bass_kernel_spmd

FP16 = mybir.dt.float16
FP32 = mybir.dt.float32
AF = mybir.ActivationFunctionType
ALU = mybir.AluOpType

NPTS = 8192          # points in x and in y
DIM = 128            # feature dim = matmul contraction K
NCORES = 8
LOCAL = NPTS // NCORES   # 1024 x-rows per core
P = 128              # partitions
CHUNKS = LOCAL // P      # 8 chunks of 128 x-rows per core
JT = 512             # j-tile width (one PSUM bank of fp32)
GROUP = 4            # j-tiles per PSUM pool buffer / ACT drain
NGRP = NPTS // (JT * GROUP)  # 4 drain groups per chunk
NBLK = NPTS // P     # 64 column blocks of 128 y-points


EBUFS = int(os.environ.get("K_EBUFS", "3"))
COLACC_GRAIN = int(os.environ.get("K_COLACC_GRAIN", "2048"))
RTREE_FINE = int(os.environ.get("K_RTREE_FINE", "0"))
SPLITCOL = int(os.environ.get("K_SPLITCOL", "0"))
SLICELOAD = int(os.environ.get("K_SLICELOAD", "0"))
REPEAT = int(os.environ.get("K_REPEAT", "1"))  # 'main' stage only


def _build_module(stage: str = "full"):
    """stage: 'full' | 'nocc' (skip collective+global) | 'main' (chunk loop
    only) | 'pe' (matmuls only) | 'pedrain' (matmuls + ACT drain)."""
    nc = bacc.Bacc(
        "TRN2",
        target_bir_lowering=False,
        debug=False,
        num_devices=NCORES,
    )

    xT2 = nc.dram_tensor("xT2", [P, LOCAL], FP16, kind="ExternalInput")
    yT = nc.dram_tensor("yT", [P, NPTS], FP16, kind="ExternalInput")
    fold_lhsT = nc.dram_tensor("fold_lhsT", [2, LOCAL], FP16, kind="ExternalInput")
    fold_rhs = nc.dram_tensor("fold_rhs", [2, NPTS], FP16, kind="ExternalInput")
    shift = nc.dram_tensor("shift", [P, 1], FP32, kind="ExternalInput")
    if stage == "host":
        # per-core partials: cols 0..63 = per-y minima (shifted space, no
        # clamp — the global min across cores happens on the host), col 64
        # = per-partition row-loss sum (clamped, final for this core's rows)
        loss = nc.dram_tensor("parts", [P, NBLK + 1], FP32,
                              kind="ExternalOutput")
        cc_in = cc_out = None
    else:
        loss = nc.dram_tensor("loss", [1, 1], FP32, kind="ExternalOutput")
        cc_in = nc.dram_tensor("cc_in", [P, NBLK + 1], FP32)
        cc_out = nc.dram_tensor("cc_out", [NCORES, P, NBLK + 1], FP32,
                                addr_space="Shared")

    with tile.TileContext(nc) as tc:
        with (
            tc.tile_pool(name="const", bufs=1) as cpool,
            tc.tile_pool(name="big", bufs=1) as bigpool,
            tc.tile_pool(name="epool", bufs=EBUFS) as epool,
            tc.tile_pool(name="scratch", bufs=1) as spool,
        ):
            sb_xT2 = cpool.tile([P, LOCAL], FP16, tag="xT2")
            sb_yT = cpool.tile([P, NPTS], FP16, tag="yT")
            sb_flhs = cpool.tile([2, LOCAL], FP16, tag="flhs")
            sb_frhs = cpool.tile([2, NPTS], FP16, tag="frhs")
            sb_shift = cpool.tile([P, 1], FP32, tag="shift")
            sb_ones = cpool.tile([P, 1], FP32, tag="ones")

            # sliced loads so early matmuls only wait on their own slice
            if SLICELOAD:
                for t in range(NPTS // JT):
                    sl = bass.ts(t, JT)
                    nc.sync.dma_start(sb_yT[:, sl], yT[:, sl])
                for c in range(CHUNKS):
                    sl = bass.ts(c, P)
                    nc.sync.dma_start(sb_xT2[:, sl], xT2[:, sl])
            else:
                nc.sync.dma_start(sb_yT[:], yT[:])
                nc.sync.dma_start(sb_xT2[:], xT2[:])
            nc.sync.dma_start(sb_flhs[:], fold_lhsT[:])
            nc.sync.dma_start(sb_frhs[:], fold_rhs[:])
            nc.sync.dma_start(sb_shift[:], shift[:])
            nc.vector.memset(sb_ones[:], 1.0)

            # two column-min accumulators: A over chunks 0-3, B over 4-7.
            # A finalizes (transpose + partition-tree) while B's chunks
            # still run, hiding half the tail.
            colaccA = bigpool.tile([P, NPTS], FP16, tag="colaccA")
            colaccB = bigpool.tile([P, NPTS], FP16, tag="colaccB")
            colaccT = bigpool.tile([P, NBLK, P], FP16, tag="colaccT")
            ctree = bigpool.tile([P, NBLK, P // 2], FP16, tag="ctree")
            cminA = spool.tile([P, NBLK], FP16, tag="cminA")
            rowmins = spool.tile([P, CHUNKS], FP32, tag="rowmins")
            rtree = spool.tile([P, NPTS // 2], FP16, tag="rtree")

            def _ctree_reduce(dst_small, b0=0, b1=NBLK):
                """Halving min-tree over colaccT's last axis for blocks
                [b0:b1]; result copied to dst_small[:, b0:b1]."""
                w = P // 2
                nc.vector.tensor_tensor(
                    ctree[:, b0:b1, 0:w], colaccT[:, b0:b1, 0:w],
                    colaccT[:, b0:b1, w:P], op=ALU.min,
                )
                while w > 1:
                    h2 = w // 2
                    nc.vector.tensor_tensor(
                        ctree[:, b0:b1, 0:h2], ctree[:, b0:b1, 0:h2],
                        ctree[:, b0:b1, h2:w], op=ALU.min,
                    )
                    w = h2
                nc.vector.tensor_copy(dst_small[:, b0:b1], ctree[:, b0:b1, 0])

            nrep = REPEAT if stage in ("main", "host") else 1
            with tc.tile_pool(name="psum", bufs=2, space="PSUM") as psum_pool:
              for rep in range(nrep):
                for c in range(CHUNKS):
                    if SPLITCOL:
                        acc_direct = colaccA if c == 0 else (
                            colaccB if c == CHUNKS // 2 else None)
                    else:
                        acc_direct = colaccA if c == 0 else None
                    if acc_direct is not None and stage not in ("pe", "pedrain"):
                        # first chunk of an accumulator: ACT drains straight
                        # into it — no separate DVE init copy needed
                        e_c = acc_direct
                    else:
                        e_c = epool.tile([P, NPTS], FP16, tag="E")
                    csl = bass.ts(c, P)
                    for g in range(NGRP):
                        pt = psum_pool.tile([P, GROUP * JT], FP32, tag="D")
                        # main matmuls of the group share one lhsT load;
                        # the K=2 bias folds share another.
                        for t in range(GROUP):
                            j0 = (g * GROUP + t) * JT
                            nc.tensor.matmul(
                                pt[:, bass.ts(t, JT)],
                                lhsT=sb_xT2[:, csl],
                                rhs=sb_yT[:, j0 : j0 + JT],
                                start=True,
                                stop=False,
                            )
                        for t in range(GROUP):
                            j0 = (g * GROUP + t) * JT
                            nc.tensor.matmul(
                                pt[:, bass.ts(t, JT)],
                                lhsT=sb_flhs[:, csl],
                                rhs=sb_frhs[:, j0 : j0 + JT],
                                start=False,
                                stop=True,
                            )
                        gsl = bass.ts(g, GROUP * JT)
                        if stage == "pe":
                            # keep a consumer so matmuls aren't dead: tiny
                            # copy of one column per group
                            nc.scalar.activation(
                                e_c[:, g : g + 1], pt[:, 0:1], AF.Copy
                            )
                        else:
                            nc.scalar.activation(e_c[:, gsl], pt[:], AF.Copy)

                    if stage in ("pe", "pedrain"):
                        # tiny reader keeps each chunk's work live
                        nc.vector.tensor_copy(
                            rowmins[0:1, c : c + 1], e_c[0:1, 0:1]
                        )
                        continue
                    # per-y minima accumulated elementwise across chunks
                    if SPLITCOL:
                        colacc = colaccA if c < CHUNKS // 2 else colaccB
                    else:
                        colacc = colaccA
                    if acc_direct is None:
                        for j0 in range(0, NPTS, COLACC_GRAIN):
                            sl = slice(j0, j0 + COLACC_GRAIN)
                            nc.vector.tensor_tensor(
                                colacc[:, sl], e_c[:, sl], colacc[:, sl],
                                op=ALU.min,
                            )
                    if (SPLITCOL and c == CHUNKS // 2 - 1
                            and stage not in ("pe", "pedrain", "main")):
                        # finalize A while B's chunks still run
                        nc.sync.dma_start_transpose(colaccT[:], colaccA[:])
                        _ctree_reduce(cminA)
                    # per-x minima: halving tree along the free axis
                    half = NPTS // 2
                    if RTREE_FINE:
                        # first level in two group-pair halves for overlap
                        q = NPTS // 4
                        nc.vector.tensor_tensor(
                            rtree[:, 0:q], e_c[:, 0:q], e_c[:, half : half + q],
                            op=ALU.min,
                        )
                        nc.vector.tensor_tensor(
                            rtree[:, q:half], e_c[:, q:half], e_c[:, half + q :],
                            op=ALU.min,
                        )
                    else:
                        nc.vector.tensor_tensor(
                            rtree[:, 0:half], e_c[:, 0:half], e_c[:, half:NPTS],
                            op=ALU.min,
                        )
                    while half > JT:
                        h2 = half // 2
                        nc.vector.tensor_tensor(
                            rtree[:, 0:h2], rtree[:, 0:h2], rtree[:, h2:half],
                            op=ALU.min,
                        )
                        half = h2
                    nc.vector.tensor_reduce(
                        rowmins[:, c : c + 1], rtree[:, 0:JT],
                        axis=mybir.AxisListType.X, op=ALU.min,
                    )
                if stage == "host":
                    _emit_tail(nc, tc, stage, spool,
                               colaccB if SPLITCOL else colaccA, colaccT,
                               ctree, cminA if SPLITCOL else None,
                               _ctree_reduce, rowmins, sb_shift, sb_ones,
                               cc_in, cc_out, loss)

            if stage in ("main", "pe", "pedrain"):
                lres0 = spool.tile([1, 1], FP32, tag="lres0")
                nc.vector.tensor_copy(lres0[:], rowmins[0:1, 0:1])
                nc.sync.dma_start(loss[:], lres0[:])
            elif stage != "host":
                _emit_tail(nc, tc, stage, spool,
                           colaccB if SPLITCOL else colaccA, colaccT, ctree,
                           cminA if SPLITCOL else None, _ctree_reduce,
                           rowmins, sb_shift, sb_ones, cc_in, cc_out, loss)

    nc.compile()
    return nc


def _emit_tail(nc, tc, stage, spool, colaccB, colaccT, ctree, cminA,
               _ctree_reduce, rowmins, sb_shift, sb_ones, cc_in, cc_out, loss):
    # rowmins -> + S, clamp 0, sum over local rows -> scalar S1_c
    rclamp = spool.tile([P, CHUNKS], FP32, tag="rclamp")
    nc.vector.tensor_scalar(
        rclamp[:], rowmins[:], sb_shift[:], 0.0, op0=ALU.add, op1=ALU.max
    )
    rsum = spool.tile([P, 1], FP32, tag="rsum")
    nc.vector.tensor_reduce(
        rsum[:], rclamp[:], axis=mybir.AxisListType.X, op=ALU.add
    )

    # finalize B's per-y minima, combine with A's.  The transpose runs in
    # two j-halves so the partition-tree on half 0 overlaps the DMA of
    # half 1.
    cminB = spool.tile([P, NBLK], FP16, tag="cminB")
    half = NPTS // 2
    hb = NBLK // 2
    nc.sync.dma_start_transpose(colaccT[:, 0:hb, :], colaccB[:, 0:half])
    nc.sync.dma_start_transpose(colaccT[:, hb:NBLK, :], colaccB[:, half:NPTS])
    _ctree_reduce(cminB, 0, hb)
    _ctree_reduce(cminB, hb, NBLK)
    colmin = spool.tile([P, NBLK], FP32, tag="colmin")
    if cminA is not None:
        nc.vector.tensor_tensor(colmin[:], cminA[:], cminB[:], op=ALU.min)
    else:
        nc.vector.tensor_copy(colmin[:], cminB[:])

    if stage == "host":
        nc.sync.dma_start(loss[:, 0:NBLK], colmin[:])
        nc.sync.dma_start(loss[:, NBLK : NBLK + 1], rsum[:])
        return

    with tc.tile_pool(name="psum2", bufs=1, space="PSUM") as pp2:
        ps_r = pp2.tile([1, 1], FP32, tag="ps_r")
        nc.tensor.matmul(ps_r[:], lhsT=rsum[:], rhs=sb_ones[:], start=True,
                         stop=True)
        s1_c = spool.tile([1, 1], FP32, tag="s1c")
        nc.scalar.activation(s1_c[:], ps_r[:], AF.Copy)

        if stage == "nocc":
            tot0 = spool.tile([1, 1], FP32, tag="tot0")
            nc.vector.tensor_tensor(tot0[:], s1_c[:], colmin[0:1, 0:1],
                                    op=ALU.add)
            nc.sync.dma_start(loss[:], tot0[:])
            return

        # exchange: [colmin fp32 | S1_c] via byte-wise AllGather
        nc.sync.dma_start(cc_in[:, 0:NBLK], colmin[:])
        nc.sync.dma_start(cc_in[0:1, NBLK : NBLK + 1], s1_c[:])
        nreps = REPEAT if stage == "fullrep" else 1
        for _ in range(nreps):
            nc.gpsimd.collective_compute(
                "AllGather",
                ALU.bypass,
                replica_groups=[list(range(NCORES))],
                ins=[cc_in[:]],
                outs=[cc_out[:]],
            )

        # ---- global reduction (identical on every core) ---------------
        gbuf = spool.tile([P, NCORES, NBLK + 1], FP32, tag="gbuf")
        nc.sync.dma_start(gbuf[:], cc_out.ap().rearrange("c p n -> p c n"))
        gmin = spool.tile([P, NCORES // 2, NBLK], FP32, tag="gmin")
        cw = NCORES // 2
        nc.vector.tensor_tensor(
            gmin[:, 0:cw, :], gbuf[:, 0:cw, 0:NBLK], gbuf[:, cw:NCORES, 0:NBLK],
            op=ALU.min,
        )
        while cw > 1:
            h2 = cw // 2
            nc.vector.tensor_tensor(
                gmin[:, 0:h2, :], gmin[:, 0:h2, :], gmin[:, h2:cw, :],
                op=ALU.min,
            )
            cw = h2
        gclamp = spool.tile([P, NBLK], FP32, tag="gclamp")
        nc.vector.tensor_scalar(
            gclamp[:], gmin[:, 0, :], sb_shift[:], 0.0, op0=ALU.add, op1=ALU.max
        )
        csum = spool.tile([P, 1], FP32, tag="csum")
        nc.vector.tensor_reduce(
            csum[:], gclamp[:], axis=mybir.AxisListType.X, op=ALU.add
        )
        ps_c = pp2.tile([1, 1], FP32, tag="ps_c")
        nc.tensor.matmul(ps_c[:], lhsT=csum[:], rhs=sb_ones[:], start=True,
                         stop=True)

        s1sum = spool.tile([1, 1], FP32, tag="s1sum")
        nc.vector.tensor_reduce(
            s1sum[:], gbuf[0:1, :, NBLK], axis=mybir.AxisListType.X, op=ALU.add
        )
        tot = spool.tile([1, 1], FP32, tag="tot")
        nc.vector.tensor_tensor(tot[:], ps_c[:], s1sum[:], op=ALU.add)
        lres = spool.tile([1, 1], FP32, tag="lres")
        nc.vector.tensor_scalar_mul(lres[:], tot[:], 1.0 / NPTS)
        nc.sync.dma_start(loss[:], lres[:])


_NC_CACHE: dict = {}


def _get_module(stage: str = "full"):
    if stage not in _NC_CACHE:
        _NC_CACHE[stage] = _build_module(stage)
    return _NC_CACHE[stage]


_RUNNER_CACHE: dict = {}


def _get_runner(stage: str = "full", donate: bool = True):
    """Build (once) a jitted SPMD callable over the 8 cores.

    Mirrors concourse.bass2jax.run_bass_via_pjrt but caches the jitted
    function so repeated calls don't re-trace, and exposes the pieces
    needed for device-resident benchmarking.
    """
    key = (stage, donate)
    if key in _RUNNER_CACHE:
        return _RUNNER_CACHE[key]

    import jax
    from jax.sharding import Mesh, PartitionSpec
    from jax.experimental.shard_map import shard_map
    import concourse.mybir as _mybir
    from concourse import bass2jax

    nc = _get_module(stage)
    bass2jax.install_neuronx_cc_hook()

    partition_name = (
        nc.partition_id_tensor.name if nc.partition_id_tensor else None
    )
    in_names: list[str] = []
    out_names: list[str] = []
    out_avals: list[jax.core.ShapedArray] = []
    zero_outs: list[np.ndarray] = []
    for alloc in nc.m.functions[0].allocations:
        if not isinstance(alloc, _mybir.MemoryLocationSet):
            continue
        name = alloc.memorylocations[0].name
        if alloc.kind == "ExternalInput":
            if name != partition_name:
                in_names.append(name)
        elif alloc.kind == "ExternalOutput":
            out_names.append(name)
            shape = tuple(alloc.tensor_shape)
            dtype = _mybir.dt.np(alloc.dtype)
            out_avals.append(jax.core.ShapedArray(shape, dtype))
            zero_outs.append(np.zeros(shape, dtype))
    n_params = len(in_names)
    n_outs = len(out_avals)
    all_names = in_names + out_names
    if partition_name is not None:
        all_names = all_names + [partition_name]

    def _body(*args):
        operands = list(args)
        if partition_name is not None:
            operands.append(bass2jax.partition_id_tensor())
        outs = bass2jax._bass_exec_p.bind(
            *operands,
            out_avals=tuple(out_avals),
            in_names=tuple(all_names),
            out_names=tuple(out_names),
            lowering_input_output_aliases=(),
            sim_require_finite=True,
            sim_require_nnan=True,
            nc=nc,
        )
        return tuple(outs)

    devices = jax.devices()[:NCORES]
    mesh = Mesh(np.asarray(devices), ("core",))
    in_specs = (PartitionSpec("core"),) * (n_params + n_outs)
    out_specs = (PartitionSpec("core"),) * n_outs
    jit_kw = (
        dict(donate_argnums=tuple(range(n_params, n_params + n_outs)))
        if donate
        else {}
    )
    sharded = jax.jit(
        shard_map(_body, mesh=mesh, in_specs=in_specs, out_specs=out_specs,
                  check_rep=False),
        keep_unused=True,
        **jit_kw,
    )
    _RUNNER_CACHE[key] = (sharded, in_names, out_names, out_avals, zero_outs,
                          mesh)
    return _RUNNER_CACHE[key]


def _run(in_maps, stage="full"):
    sharded, in_names, out_names, out_avals, zero_outs, _ = _get_runner(stage)
    concat_in = [
        np.concatenate([np.asarray(in_maps[c][n]) for c in range(NCORES)], axis=0)
        for n in in_names
    ]
    concat_zeros = [
        np.zeros((NCORES * z.shape[0], *z.shape[1:]), z.dtype) for z in zero_outs
    ]
    out_arrs = sharded(*concat_in, *concat_zeros)
    return [
        {
            n: np.asarray(out_arrs[i]).reshape(NCORES, *out_avals[i].shape)[c]
            for i, n in enumerate(out_names)
        }
        for c in range(NCORES)
    ]


def _prep_inputs(x: np.ndarray, y: np.ndarray):
    return _prep_inputs_s(x, y)[0]


def _prep_inputs_s(x: np.ndarray, y: np.ndarray):
    x = np.asarray(x, np.float32)
    y = np.asarray(y, np.float32)
    x2 = np.sum(x.astype(np.float64) ** 2, axis=1)
    y2 = np.sum(y.astype(np.float64) ** 2, axis=1)
    s = float(x2.min() + y2.min())
    x2s = (x2 - x2.min()).astype(np.float32)
    y2s = (y2 - y2.min()).astype(np.float32)

    yT = np.ascontiguousarray(y.T).astype(np.float16)
    fold_rhs = np.empty((2, NPTS), np.float16)
    fold_rhs[0] = y2s.astype(np.float16)
    fold_rhs[1] = 1.0
    shift = np.full((P, 1), s, np.float32)

    in_maps = []
    for c in range(NCORES):
        sl = slice(c * LOCAL, (c + 1) * LOCAL)
        xT2 = np.ascontiguousarray((-2.0 * x[sl]).T).astype(np.float16)
        fold_lhsT = np.empty((2, LOCAL), np.float16)
        fold_lhsT[0] = 1.0
        fold_lhsT[1] = x2s[sl].astype(np.float16)
        in_maps.append(
            {
                "xT2": xT2,
                "yT": yT,
                "fold_lhsT": fold_lhsT,
                "fold_rhs": fold_rhs,
                "shift": shift,
            }
        )
    return in_maps, s


def kernel(x: np.ndarray, y: np.ndarray, **_ignored):
    x = np.asarray(x, np.float32)
    y = np.asarray(y, np.float32)
    in_maps, s = _prep_inputs_s(x, y)
    results = _run(in_maps, stage="host")
    parts = np.stack([results[c]["parts"] for c in range(NCORES)])  # [8,128,65]
    colmin = parts[:, :, 0:NBLK].min(axis=0)       # global per-y minima
    s2 = np.maximum(colmin.astype(np.float64) + s, 0.0).sum()
    s1 = parts[:, :, NBLK].astype(np.float64).sum()
    return np.float32((s1 + s2) / NPTS)


# revision 53
# speedup vs baseline: 43.9937x; 1.1795x over previous
"""Chamfer loss kernel for Trainium2 (8 NeuronCores, Bass/Tile).

Problem: x, y of shape [8192, 128] fp32.
  dist[i,j] = max(||x_i||^2 + ||y_j||^2 - 2 x_i.y_j, 0)
  loss = (sum_j min_i dist[i,j] + sum_i min_j dist[i,j]) / 8192

Sharding: x rows are split across the 8 cores (1024 rows each); every core
holds all of y. Each core computes its [1024, 8192] distance tile via PE
matmuls (K = 128 features on partitions):

  PSUM tile = (-2 x_chunk^T)^T @ y^T            (K=128 fp16 matmul)
            + [ones; x2_chunk]^T @ [y2; ones]   (K=2 rank-2 bias fold)
  => PSUM[i, j] = shifted dist (x2/y2 are shifted by their minima; the
     shift S is added back to the final [128]-sized min vectors, which
     keeps the fp16 bias rows small and precise).

ScalarE drains PSUM -> SBUF fp16 "E" tiles (one pass). VectorE then does
both reductions as 2x-mode fp16 tensor-tensor mins:
  - per-x row minima: pairwise halving tree along the free axis
  - per-y col minima: elementwise min accumulated across the 8 x-chunks
    (cross-partition reduction deferred to a DMA-transpose + halving tree)
Finally a byte-wise AllGather exchanges each core's per-y minima vector
[128,64] plus its local row-loss partial; every core reduces the gathered
data to the final scalar loss.
"""

import os
import sys

import numpy as np

sys.path.insert(0, "/opt/trn_rl_repo")
os.environ.setdefault("MYCRO_LOCAL_CACHE", "1")

import concourse.bass as bass
import concourse.bacc as bacc
import concourse.mybir as mybir
import concourse.tile as tile
from concourse.bass_utils import run_bass_kernel_spmd

FP16 = mybir.dt.float16
FP32 = mybir.dt.float32
AF = mybir.ActivationFunctionType
ALU = mybir.AluOpType

NPTS = 8192          # points in x and in y
DIM = 128            # feature dim = matmul contraction K
NCORES = 8
LOCAL = NPTS // NCORES   # 1024 x-rows per core
P = 128              # partitions
CHUNKS = LOCAL // P      # 8 chunks of 128 x-rows per core
JT = 512             # j-tile width (one PSUM bank of fp32)
GROUP = 4            # j-tiles per PSUM pool buffer / ACT drain
NGRP = NPTS // (JT * GROUP)  # 4 drain groups per chunk
NBLK = NPTS // P     # 64 column blocks of 128 y-points


EBUFS = int(os.environ.get("K_EBUFS", "3"))
COLACC_GRAIN = int(os.environ.get("K_COLACC_GRAIN", "2048"))
RTREE_FINE = int(os.environ.get("K_RTREE_FINE", "0"))
SPLITCOL = int(os.environ.get("K_SPLITCOL", "0"))
SLICELOAD = int(os.environ.get("K_SLICELOAD", "0"))
REPEAT = int(os.environ.get("K_REPEAT", "1"))  # 'main' stage only


def _build_module(stage: str = "full"):
    """stage: 'full' | 'nocc' (skip collective+global) | 'main' (chunk loop
    only) | 'pe' (matmuls only) | 'pedrain' (matmuls + ACT drain) | 'host'
    (per-core partials, host combine; 'hostN' = body repeated N times)."""
    nrep_override = None
    if stage.startswith("host") and len(stage) > 4:
        nrep_override = int(stage[4:])
        stage = "host"
    nc = bacc.Bacc(
        "TRN2",
        target_bir_lowering=False,
        debug=False,
        num_devices=NCORES,
    )

    xT2 = nc.dram_tensor("xT2", [P, LOCAL], FP16, kind="ExternalInput")
    yT = nc.dram_tensor("yT", [P, NPTS], FP16, kind="ExternalInput")
    fold_lhsT = nc.dram_tensor("fold_lhsT", [2, LOCAL], FP16, kind="ExternalInput")
    fold_rhs = nc.dram_tensor("fold_rhs", [2, NPTS], FP16, kind="ExternalInput")
    shift = nc.dram_tensor("shift", [P, 1], FP32, kind="ExternalInput")
    if stage == "host":
        # per-core partials: cols 0..63 = per-y minima (shifted space, no
        # clamp — the global min across cores happens on the host), col 64
        # = per-partition row-loss sum (clamped, final for this core's rows)
        loss = nc.dram_tensor("parts", [P, NBLK + 1], FP32,
                              kind="ExternalOutput")
        cc_in = cc_out = None
    else:
        loss = nc.dram_tensor("loss", [1, 1], FP32, kind="ExternalOutput")
        cc_in = nc.dram_tensor("cc_in", [P, NBLK + 1], FP32)
        cc_out = nc.dram_tensor("cc_out", [NCORES, P, NBLK + 1], FP32,
                                addr_space="Shared")

    with tile.TileContext(nc) as tc:
        with (
            tc.tile_pool(name="const", bufs=1) as cpool,
            tc.tile_pool(name="big", bufs=1) as bigpool,
            tc.tile_pool(name="epool", bufs=EBUFS) as epool,
            tc.tile_pool(name="scratch", bufs=1) as spool,
        ):
            sb_xT2 = cpool.tile([P, LOCAL], FP16, tag="xT2")
            sb_yT = cpool.tile([P, NPTS], FP16, tag="yT")
            sb_flhs = cpool.tile([2, LOCAL], FP16, tag="flhs")
            sb_frhs = cpool.tile([2, NPTS], FP16, tag="frhs")
            sb_shift = cpool.tile([P, 1], FP32, tag="shift")
            sb_ones = cpool.tile([P, 1], FP32, tag="ones")

            # sliced loads so early matmuls only wait on their own slice
            if SLICELOAD:
                for t in range(NPTS // JT):
                    sl = bass.ts(t, JT)
                    nc.sync.dma_start(sb_yT[:, sl], yT[:, sl])
                for c in range(CHUNKS):
                    sl = bass.ts(c, P)
                    nc.sync.dma_start(sb_xT2[:, sl], xT2[:, sl])
            else:
                nc.sync.dma_start(sb_yT[:], yT[:])
                nc.sync.dma_start(sb_xT2[:], xT2[:])
            nc.sync.dma_start(sb_flhs[:], fold_lhsT[:])
            nc.sync.dma_start(sb_frhs[:], fold_rhs[:])
            nc.sync.dma_start(sb_shift[:], shift[:])
            nc.vector.memset(sb_ones[:], 1.0)

            # two column-min accumulators: A over chunks 0-3, B over 4-7.
            # A finalizes (transpose + partition-tree) while B's chunks
            # still run, hiding half the tail.
            colaccA = bigpool.tile([P, NPTS], FP16, tag="colaccA")
            colaccB = bigpool.tile([P, NPTS], FP16, tag="colaccB")
            colaccT = bigpool.tile([P, NBLK, P], FP16, tag="colaccT")
            ctree = bigpool.tile([P, NBLK, P // 2], FP16, tag="ctree")
            cminA = spool.tile([P, NBLK], FP16, tag="cminA")
            rowmins = spool.tile([P, CHUNKS], FP32, tag="rowmins")
            rtree = spool.tile([P, NPTS // 2], FP16, tag="rtree")

            def _ctree_reduce(dst_small, b0=0, b1=NBLK):
                """Halving min-tree over colaccT's last axis for blocks
                [b0:b1]; result copied to dst_small[:, b0:b1]."""
                w = P // 2
                nc.vector.tensor_tensor(
                    ctree[:, b0:b1, 0:w], colaccT[:, b0:b1, 0:w],
                    colaccT[:, b0:b1, w:P], op=ALU.min,
                )
                while w > 1:
                    h2 = w // 2
                    nc.vector.tensor_tensor(
                        ctree[:, b0:b1, 0:h2], ctree[:, b0:b1, 0:h2],
                        ctree[:, b0:b1, h2:w], op=ALU.min,
                    )
                    w = h2
                nc.vector.tensor_copy(dst_small[:, b0:b1], ctree[:, b0:b1, 0])

            if nrep_override is not None:
                nrep = nrep_override
            elif stage in ("main", "host"):
                nrep = REPEAT
            else:
                nrep = 1
            with tc.tile_pool(name="psum", bufs=2, space="PSUM") as psum_pool:
              for rep in range(nrep):
                for c in range(CHUNKS):
                    if SPLITCOL:
                        acc_direct = colaccA if c == 0 else (
                            colaccB if c == CHUNKS // 2 else None)
                    else:
                        acc_direct = colaccA if c == 0 else None
                    if acc_direct is not None and stage not in ("pe", "pedrain"):
                        # first chunk of an accumulator: ACT drains straight
                        # into it — no separate DVE init copy needed
                        e_c = acc_direct
                    else:
                        e_c = epool.tile([P, NPTS], FP16, tag="E")
                    csl = bass.ts(c, P)
                    for g in range(NGRP):
                        pt = psum_pool.tile([P, GROUP * JT], FP32, tag="D")
                        # main matmuls of the group share one lhsT load;
                        # the K=2 bias folds share another.
                        for t in range(GROUP):
                            j0 = (g * GROUP + t) * JT
                            nc.tensor.matmul(
                                pt[:, bass.ts(t, JT)],
                                lhsT=sb_xT2[:, csl],
                                rhs=sb_yT[:, j0 : j0 + JT],
                                start=True,
                                stop=False,
                            )
                        for t in range(GROUP):
                            j0 = (g * GROUP + t) * JT
                            nc.tensor.matmul(
                                pt[:, bass.ts(t, JT)],
                                lhsT=sb_flhs[:, csl],
                                rhs=sb_frhs[:, j0 : j0 + JT],
                                start=False,
                                stop=True,
                            )
                        gsl = bass.ts(g, GROUP * JT)
                        if stage == "pe":
                            # keep a consumer so matmuls aren't dead: tiny
                            # copy of one column per group
                            nc.scalar.activation(
                                e_c[:, g : g + 1], pt[:, 0:1], AF.Copy
                            )
                        else:
                            nc.scalar.activation(e_c[:, gsl], pt[:], AF.Copy)

                    if stage in ("pe", "pedrain"):
                        # tiny reader keeps each chunk's work live
                        nc.vector.tensor_copy(
                            rowmins[0:1, c : c + 1], e_c[0:1, 0:1]
                        )
                        continue
                    # per-y minima accumulated elementwise across chunks
                    if SPLITCOL:
                        colacc = colaccA if c < CHUNKS // 2 else colaccB
                    else:
                        colacc = colaccA
                    if acc_direct is None:
                        for j0 in range(0, NPTS, COLACC_GRAIN):
                            sl = slice(j0, j0 + COLACC_GRAIN)
                            nc.vector.tensor_tensor(
                                colacc[:, sl], e_c[:, sl], colacc[:, sl],
                                op=ALU.min,
                            )
                    if (SPLITCOL and c == CHUNKS // 2 - 1
                            and stage not in ("pe", "pedrain", "main")):
                        # finalize A while B's chunks still run
                        nc.sync.dma_start_transpose(colaccT[:], colaccA[:])
                        _ctree_reduce(cminA)
                    # per-x minima: halving tree along the free axis
                    half = NPTS // 2
                    if RTREE_FINE:
                        # first level in two group-pair halves for overlap
                        q = NPTS // 4
                        nc.vector.tensor_tensor(
                            rtree[:, 0:q], e_c[:, 0:q], e_c[:, half : half + q],
                            op=ALU.min,
                        )
                        nc.vector.tensor_tensor(
                            rtree[:, q:half], e_c[:, q:half], e_c[:, half + q :],
                            op=ALU.min,
                        )
                    else:
                        nc.vector.tensor_tensor(
                            rtree[:, 0:half], e_c[:, 0:half], e_c[:, half:NPTS],
                            op=ALU.min,
                        )
                    while half > JT:
                        h2 = half // 2
                        nc.vector.tensor_tensor(
                            rtree[:, 0:h2], rtree[:, 0:h2], rtree[:, h2:half],
                            op=ALU.min,
                        )
                        half = h2
                    nc.vector.tensor_reduce(
                        rowmins[:, c : c + 1], rtree[:, 0:JT],
                        axis=mybir.AxisListType.X, op=ALU.min,
                    )
                if stage == "host":
                    _emit_tail(nc, tc, stage, spool,
                               colaccB if SPLITCOL else colaccA, colaccT,
                               ctree, cminA if SPLITCOL else None,
                               _ctree_reduce, rowmins, sb_shift, sb_ones,
                               cc_in, cc_out, loss)

            if stage in ("main", "pe", "pedrain"):
                lres0 = spool.tile([1, 1], FP32, tag="lres0")
                nc.vector.tensor_copy(lres0[:], rowmins[0:1, 0:1])
                nc.sync.dma_start(loss[:], lres0[:])
            elif stage != "host":
                _emit_tail(nc, tc, stage, spool,
                           colaccB if SPLITCOL else colaccA, colaccT, ctree,
                           cminA if SPLITCOL else None, _ctree_reduce,
                           rowmins, sb_shift, sb_ones, cc_in, cc_out, loss)

    nc.compile()
    return nc


def _emit_tail(nc, tc, stage, spool, colaccB, colaccT, ctree, cminA,
               _ctree_reduce, rowmins, sb_shift, sb_ones, cc_in, cc_out, loss):
    # rowmins -> + S, clamp 0, sum over local rows -> scalar S1_c
    rclamp = spool.tile([P, CHUNKS], FP32, tag="rclamp")
    nc.vector.tensor_scalar(
        rclamp[:], rowmins[:], sb_shift[:], 0.0, op0=ALU.add, op1=ALU.max
    )
    rsum = spool.tile([P, 1], FP32, tag="rsum")
    nc.vector.tensor_reduce(
        rsum[:], rclamp[:], axis=mybir.AxisListType.X, op=ALU.add
    )

    # finalize B's per-y minima, combine with A's.  The transpose runs in
    # two j-halves so the partition-tree on half 0 overlaps the DMA of
    # half 1.
    cminB = spool.tile([P, NBLK], FP16, tag="cminB")
    half = NPTS // 2
    hb = NBLK // 2
    nc.sync.dma_start_transpose(colaccT[:, 0:hb, :], colaccB[:, 0:half])
    nc.sync.dma_start_transpose(colaccT[:, hb:NBLK, :], colaccB[:, half:NPTS])
    _ctree_reduce(cminB, 0, hb)
    _ctree_reduce(cminB, hb, NBLK)
    colmin = spool.tile([P, NBLK], FP32, tag="colmin")
    if cminA is not None:
        nc.vector.tensor_tensor(colmin[:], cminA[:], cminB[:], op=ALU.min)
    else:
        nc.vector.tensor_copy(colmin[:], cminB[:])

    if stage == "host":
        nc.sync.dma_start(loss[:, 0:NBLK], colmin[:])
        nc.sync.dma_start(loss[:, NBLK : NBLK + 1], rsum[:])
        return

    with tc.tile_pool(name="psum2", bufs=1, space="PSUM") as pp2:
        ps_r = pp2.tile([1, 1], FP32, tag="ps_r")
        nc.tensor.matmul(ps_r[:], lhsT=rsum[:], rhs=sb_ones[:], start=True,
                         stop=True)
        s1_c = spool.tile([1, 1], FP32, tag="s1c")
        nc.scalar.activation(s1_c[:], ps_r[:], AF.Copy)

        if stage == "nocc":
            tot0 = spool.tile([1, 1], FP32, tag="tot0")
            nc.vector.tensor_tensor(tot0[:], s1_c[:], colmin[0:1, 0:1],
                                    op=ALU.add)
            nc.sync.dma_start(loss[:], tot0[:])
            return

        # exchange: [colmin fp32 | S1_c] via byte-wise AllGather
        nc.sync.dma_start(cc_in[:, 0:NBLK], colmin[:])
        nc.sync.dma_start(cc_in[0:1, NBLK : NBLK + 1], s1_c[:])
        nreps = REPEAT if stage == "fullrep" else 1
        for _ in range(nreps):
            nc.gpsimd.collective_compute(
                "AllGather",
                ALU.bypass,
                replica_groups=[list(range(NCORES))],
                ins=[cc_in[:]],
                outs=[cc_out[:]],
            )

        # ---- global reduction (identical on every core) ---------------
        gbuf = spool.tile([P, NCORES, NBLK + 1], FP32, tag="gbuf")
        nc.sync.dma_start(gbuf[:], cc_out.ap().rearrange("c p n -> p c n"))
        gmin = spool.tile([P, NCORES // 2, NBLK], FP32, tag="gmin")
        cw = NCORES // 2
        nc.vector.tensor_tensor(
            gmin[:, 0:cw, :], gbuf[:, 0:cw, 0:NBLK], gbuf[:, cw:NCORES, 0:NBLK],
            op=ALU.min,
        )
        while cw > 1:
            h2 = cw // 2
            nc.vector.tensor_tensor(
                gmin[:, 0:h2, :], gmin[:, 0:h2, :], gmin[:, h2:cw, :],
                op=ALU.min,
            )
            cw = h2
        gclamp = spool.tile([P, NBLK], FP32, tag="gclamp")
        nc.vector.tensor_scalar(
            gclamp[:], gmin[:, 0, :], sb_shift[:], 0.0, op0=ALU.add, op1=ALU.max
        )
        csum = spool.tile([P, 1], FP32, tag="csum")
        nc.vector.tensor_reduce(
            csum[:], gclamp[:], axis=mybir.AxisListType.X, op=ALU.add
        )
        ps_c = pp2.tile([1, 1], FP32, tag="ps_c")
        nc.tensor.matmul(ps_c[:], lhsT=csum[:], rhs=sb_ones[:], start=True,
                         stop=True)

        s1sum = spool.tile([1, 1], FP32, tag="s1sum")
        nc.vector.tensor_reduce(
            s1sum[:], gbuf[0:1, :, NBLK], axis=mybir.AxisListType.X, op=ALU.add
        )
        tot = spool.tile([1, 1], FP32, tag="tot")
        nc.vector.tensor_tensor(tot[:], ps_c[:], s1sum[:], op=ALU.add)
        lres = spool.tile([1, 1], FP32, tag="lres")
        nc.vector.tensor_scalar_mul(lres[:], tot[:], 1.0 / NPTS)
        nc.sync.dma_start(loss[:], lres[:])


_NC_CACHE: dict = {}


def _get_module(stage: str = "full"):
    if stage not in _NC_CACHE:
        _NC_CACHE[stage] = _build_module(stage)
    return _NC_CACHE[stage]


_RUNNER_CACHE: dict = {}


def _get_runner(stage: str = "full", donate: bool = True):
    """Build (once) a jitted SPMD callable over the 8 cores.

    Mirrors concourse.bass2jax.run_bass_via_pjrt but caches the jitted
    function so repeated calls don't re-trace, and exposes the pieces
    needed for device-resident benchmarking.
    """
    key = (stage, donate)
    if key in _RUNNER_CACHE:
        return _RUNNER_CACHE[key]

    import jax
    from jax.sharding import Mesh, PartitionSpec
    from jax.experimental.shard_map import shard_map
    import concourse.mybir as _mybir
    from concourse import bass2jax

    nc = _get_module(stage)
    bass2jax.install_neuronx_cc_hook()

    partition_name = (
        nc.partition_id_tensor.name if nc.partition_id_tensor else None
    )
    in_names: list[str] = []
    out_names: list[str] = []
    out_avals: list[jax.core.ShapedArray] = []
    zero_outs: list[np.ndarray] = []
    for alloc in nc.m.functions[0].allocations:
        if not isinstance(alloc, _mybir.MemoryLocationSet):
            continue
        name = alloc.memorylocations[0].name
        if alloc.kind == "ExternalInput":
            if name != partition_name:
                in_names.append(name)
        elif alloc.kind == "ExternalOutput":
            out_names.append(name)
            shape = tuple(alloc.tensor_shape)
            dtype = _mybir.dt.np(alloc.dtype)
            out_avals.append(jax.core.ShapedArray(shape, dtype))
            zero_outs.append(np.zeros(shape, dtype))
    n_params = len(in_names)
    n_outs = len(out_avals)
    all_names = in_names + out_names
    if partition_name is not None:
        all_names = all_names + [partition_name]

    def _body(*args):
        operands = list(args)
        if partition_name is not None:
            operands.append(bass2jax.partition_id_tensor())
        outs = bass2jax._bass_exec_p.bind(
            *operands,
            out_avals=tuple(out_avals),
            in_names=tuple(all_names),
            out_names=tuple(out_names),
            lowering_input_output_aliases=(),
            sim_require_finite=True,
            sim_require_nnan=True,
            nc=nc,
        )
        return tuple(outs)

    devices = jax.devices()[:NCORES]
    mesh = Mesh(np.asarray(devices), ("core",))
    in_specs = (PartitionSpec("core"),) * (n_params + n_outs)
    out_specs = (PartitionSpec("core"),) * n_outs
    jit_kw = (
        dict(donate_argnums=tuple(range(n_params, n_params + n_outs)))
        if donate
        else {}
    )
    sharded = jax.jit(
        shard_map(_body, mesh=mesh, in_specs=in_specs, out_specs=out_specs,
                  check_rep=False),
        keep_unused=True,
        **jit_kw,
    )
    _RUNNER_CACHE[key] = (sharded, in_names, out_names, out_avals, zero_outs,
                          mesh)
    return _RUNNER_CACHE[key]


def _run(in_maps, stage="full"):
    sharded, in_names, out_names, out_avals, zero_outs, _ = _get_runner(stage)
    concat_in = [
        np.concatenate([np.asarray(in_maps[c][n]) for c in range(NCORES)], axis=0)
        for n in in_names
    ]
    concat_zeros = [
        np.zeros((NCORES * z.shape[0], *z.shape[1:]), z.dtype) for z in zero_outs
    ]
    out_arrs = sharded(*concat_in, *concat_zeros)
    return [
        {
            n: np.asarray(out_arrs[i]).reshape(NCORES, *out_avals[i].shape)[c]
            for i, n in enumerate(out_names)
        }
        for c in range(NCORES)
    ]


def _prep_inputs(x: np.ndarray, y: np.ndarray):
    return _prep_inputs_s(x, y)[0]


def _prep_inputs_s(x: np.ndarray, y: np.ndarray):
    x = np.asarray(x, np.float32)
    y = np.asarray(y, np.float32)
    x2 = np.sum(x.astype(np.float64) ** 2, axis=1)
    y2 = np.sum(y.astype(np.float64) ** 2, axis=1)
    s = float(x2.min() + y2.min())
    x2s = (x2 - x2.min()).astype(np.float32)
    y2s = (y2 - y2.min()).astype(np.float32)

    yT = np.ascontiguousarray(y.T).astype(np.float16)
    fold_rhs = np.empty((2, NPTS), np.float16)
    fold_rhs[0] = y2s.astype(np.float16)
    fold_rhs[1] = 1.0
    shift = np.full((P, 1), s, np.float32)

    in_maps = []
    for c in range(NCORES):
        sl = slice(c * LOCAL, (c + 1) * LOCAL)
        xT2 = np.ascontiguousarray((-2.0 * x[sl]).T).astype(np.float16)
        fold_lhsT = np.empty((2, LOCAL), np.float16)
        fold_lhsT[0] = 1.0
        fold_lhsT[1] = x2s[sl].astype(np.float16)
        in_maps.append(
            {
                "xT2": xT2,
                "yT": yT,
                "fold_lhsT": fold_lhsT,
                "fold_rhs": fold_rhs,
                "shift": shift,
            }
        )
    return in_maps, s


def kernel(x: np.ndarray, y: np.ndarray, **_ignored):
    x = np.asarray(x, np.float32)
    y = np.asarray(y, np.float32)
    in_maps, s = _prep_inputs_s(x, y)
    results = _run(in_maps, stage="host")
    parts = np.stack([results[c]["parts"] for c in range(NCORES)])  # [8,128,65]
    colmin = parts[:, :, 0:NBLK].min(axis=0)       # global per-y minima
    s2 = np.maximum(colmin.astype(np.float64) + s, 0.0).sum()
    s1 = parts[:, :, NBLK].astype(np.float64).sum()
    return np.float32((s1 + s2) / NPTS)
